# revision 11
# baseline (speedup 1.0000x reference)
"""LocalCorrelation (13x13 cost volume) Trainium2 kernel.

Full inputs z_t, z_t1: [8, 256, 128, 128] f32 -> out [8, 169, 128, 128] f32.
out[b, 13*di+dj, h, w] = sum_c z_t[b,c,h,w] * pad(z_t1)[b,c,h+di,w+dj] / 16

Sharding: data-parallel over batch, 1 batch element per NeuronCore (8 cores).

Per-core algorithm (SPMD, identical program):
  - Load z_t (scaled by 1/16) and zero-padded z_t1 into SBUF as bf16,
    channel dim on partitions (2 chunks of 128).
  - For each 8x16 output-pixel block: TensorE "block gram" matmuls
    stationary = z_t block [c,128 pixels], streaming = padded z_t1
    20x28 window [c,560] -> PSUM f32 (accumulated over 2 c-chunks).
  - PSUM -> SBUF bf16, dense DMA to DRAM scratch.
  - Band-read DMAs (per stripe,dh): each pixel's 349-elem contiguous
    window span (covers all 169 taps at stride-28/1) comes back with
    506B+ runs -- the per-pixel diagonal offset is absorbed by flat
    DRAM addressing, 8 DMAs/stripe instead of 104.
  - TensorE transposes flip [px, tap] -> [tap, px] (taps 0:117, 117:169
    via strided stationary APs directly on the padded band).
  - PSUM -> SBUF f32 assembly into [tap, (dh, w)] stripes; 2 dense
    512B-run output DMAs per stripe.
"""

import numpy as np

C = 256
H = W = 128
KS = 13
KK = 169
RAD = 6
HP = WP = 140  # padded spatial
SA = 8  # block rows (stripe height)
SB = 16  # block cols
NWB = W // SB  # 8 w-blocks per stripe
NST = H // SA  # 16 stripes
WINP = SA + 2 * RAD  # 20 streamed rows per window
WINQ = SB + 2 * RAD  # 28 streamed cols per window
WIN = WINP * WINQ  # 560
BSPAN = 12 * WINQ + KS  # 349: contiguous span covering all taps of one px
BPAD = KS * WINQ  # 364: padded band stride (13 x 28) for strided views
TA = 9 * KS  # 117 taps in first transpose chunk (di 0..8)
TB = 4 * KS  # 52 taps in second chunk (di 9..12)

_cache = {}


def _build():
    import concourse.bass as bass
    import concourse.mybir as mybir
    import concourse.tile as tile
    from concourse import bacc
    from concourse.masks import make_identity

    f32 = mybir.dt.float32
    bf16 = mybir.dt.bfloat16

    nc = bacc.Bacc("TRN2", target_bir_lowering=False, debug=False)
    zt_d = nc.dram_tensor("z_t", [C, H, W], f32, kind="ExternalInput")
    z1_d = nc.dram_tensor("z_t1", [C, H, W], f32, kind="ExternalInput")
    out_d = nc.dram_tensor("out", [KK, H, W], f32, kind="ExternalOutput")

    with tile.TileContext(nc) as tc:
        with tc.tile_pool(name="persist", bufs=1) as pp:
            ZT = [pp.tile([128, H * W], bf16, tag=f"zt{k}", name=f"zt{k}") for k in range(2)]
            Z1P = [pp.tile([128, HP * WP], bf16, tag=f"z1p{k}", name=f"z1p{k}") for k in range(2)]
            IDN = pp.tile([128, 128], bf16, tag="idn", name="idn")
            make_identity(nc, IDN[:, :])

            # ---- input load: cast f32->bf16 via SWDGE DMA ----
            # ZT is stored BLOCK-MAJOR: free index = ((si*8 + wb)*8 + dh)*16 + dw
            # so each 8x16 block's 128 pixels are contiguous (matmul stationary
            # operand requires a single free dim).
            # memset only the pad border of Z1P (not the interior, which the
            # slab copies overwrite anyway).
            for k in range(2):
                zv = Z1P[k].rearrange("c (h w) -> c h w", h=HP)
                nc.vector.memset(zv[:, 0:RAD, :], 0.0)
                nc.vector.memset(zv[:, HP - RAD:HP, :], 0.0)
                nc.vector.memset(zv[:, RAD:HP - RAD, 0:RAD], 0.0)
                nc.vector.memset(zv[:, RAD:HP - RAD, WP - RAD:WP], 0.0)

            with tc.tile_pool(name="ld", bufs=2) as ldp:
                # z1 first: Z1P must be fully written before stripe 0's
                # matmuls can fire (whole-tile dep tracking), zt streams after
                for s in range(4):  # 32-row slabs
                    for k in range(2):
                        z1u = ldp.tile([128, 32 * W], bf16, tag="z1u", name="z1u")
                        src = z1_d.ap()[k * 128:(k + 1) * 128, s * 32:(s + 1) * 32, :]
                        nc.gpsimd.dma_start(
                            z1u.rearrange("c (h w) -> c h w", h=32), src)
                        dst = Z1P[k].rearrange("c (h w) -> c h w", h=HP)[
                            :, RAD + s * 32: RAD + (s + 1) * 32, RAD: RAD + W]
                        nc.vector.tensor_copy(dst, z1u.rearrange("c (h w) -> c h w", h=32))
                for s in range(4):  # 32-row slabs
                    for k in range(2):
                        ztu = ldp.tile([128, 32 * W], bf16, tag="ztu", name="ztu")
                        src = zt_d.ap()[k * 128:(k + 1) * 128, s * 32:(s + 1) * 32, :]
                        nc.gpsimd.dma_start(
                            ztu.rearrange("c (h w) -> c h w", h=32), src)
                        for sl in range(4):
                            si_g = s * 4 + sl
                            srcv = ztu.rearrange(
                                "c (h wb dw) -> c wb h dw", h=32, wb=NWB)[
                                :, :, sl * SA:(sl + 1) * SA, :]
                            dstv = ZT[k][:, si_g * 1024:(si_g + 1) * 1024].rearrange(
                                "c (wb dh dw) -> c wb dh dw", wb=NWB, dh=SA)
                            nc.scalar.mul(dstv, srcv, 1.0 / 16.0)

            # ---- main loop ----
            with (
                tc.tile_pool(name="xbp", bufs=2) as xbp,
                tc.tile_pool(name="bnp", bufs=3) as bnp,
                tc.tile_pool(name="o2p", bufs=2) as o2p,
                tc.tile_pool(name="o3p", bufs=2) as o3p,
                tc.tile_pool(name="psp", bufs=2, space="PSUM") as psp,
                tc.tile_pool(name="tpp", bufs=2, space="PSUM") as tpp,
                tc.tile_pool(name="scrp", bufs=3, space="DRAM") as scrp,
            ):
                for si in range(NST):
                    h0 = si * SA
                    scr = scrp.tile([128, NWB, WIN], bf16, tag="scr", name="scr")
                    xb = xbp.tile([128, NWB * WIN], bf16, tag="xb", name="xb")
                    for wb in range(NWB):
                        w0 = wb * SB
                        ps = [psp.tile([128, 280], f32, tag=f"ps{i}", name=f"ps{i}")
                              for i in range(2)]
                        for k in range(2):
                            blk = si * NWB + wb
                            lhsT = ZT[k][:, blk * 128:(blk + 1) * 128]
                            for half in range(2):
                                rhs = Z1P[k].rearrange("c (h w) -> c h w", h=HP)[
                                    :, h0 + 10 * half: h0 + 10 * (half + 1),
                                    w0:w0 + WINQ]
                                nc.tensor.matmul(ps[half][:, :], lhsT, rhs,
                                                 start=(k == 0), stop=(k == 1))
                        for half in range(2):
                            dst = xb[:, wb * WIN + half * 280: wb * WIN + (half + 1) * 280]
                            if wb % 2 == 0:
                                nc.scalar.copy(dst, ps[half][:, :])
                            else:
                                nc.vector.tensor_copy(dst, ps[half][:, :])

                    # dense scratch write (8960B contiguous per px)
                    scr_w = bass.AP(scr.tensor, 0, [[NWB * WIN, 128], [WIN, NWB], [1, WIN]])
                    nc.sync.dma_start(scr_w, xb.rearrange("p (wb s) -> p wb s", wb=NWB))

                    # band read: per dh, pull each pixel's 349-elem contiguous
                    # span (taps live at di*28+dj inside it); per-pixel offset
                    # dh*28+dw is absorbed by flat DRAM addressing.
                    bnd = bnp.tile([128, NWB * BPAD], bf16, tag="bnd", name="bnd")
                    for dh in range(SA):
                        src = bass.AP(scr.tensor, dh * (SB * NWB * WIN + WINQ),
                                      [[NWB * WIN + 1, SB],
                                       [WIN, NWB],
                                       [1, BSPAN]])
                        dst = bnd[dh * SB:(dh + 1) * SB, :].rearrange(
                            "p (wb kk) -> p wb kk", wb=NWB)[:, :, 0:BSPAN]
                        nc.sync.dma_start(dst, src)

                    # compact taps (di*28+dj) out of the band into contiguous
                    # [px, (wb, tap)] -- matmul stationary APs must be 1-D.
                    o2v = o2p.tile([128, NWB * KK], bf16, tag="o2v", name="o2v")
                    csrc = bnd.rearrange("p (wb di dj) -> p wb di dj", wb=NWB, di=KS)[
                        :, :, :, 0:KS]
                    cdst = o2v.rearrange("p (wb di dj) -> p wb di dj", wb=NWB, di=KS)
                    if si % 2 == 0:
                        nc.vector.tensor_copy(cdst, csrc)
                    else:
                        nc.scalar.copy(cdst, csrc)

                    # TensorE transpose [px, tap] -> [tap, px]
                    o3a = o3p.tile([TA, SA * W], f32, tag="o3a", name="o3a")
                    o3b = o3p.tile([TB, SA * W], f32, tag="o3b", name="o3b")
                    for wb in range(NWB):
                        tpa = tpp.tile([TA, 128], bf16, tag="tpa", name="tpa")
                        tpb = tpp.tile([TB, 128], bf16, tag="tpb", name="tpb")
                        nc.tensor.transpose(
                            tpa[:, :], o2v[:, wb * KK: wb * KK + TA], IDN[:, :])
                        nc.tensor.transpose(
                            tpb[:, :], o2v[:, wb * KK + TA: (wb + 1) * KK], IDN[:, :])
                        dsta = o3a.rearrange("t (dh w) -> t dh w", dh=SA)[
                            :, :, wb * SB:(wb + 1) * SB]
                        dstb = o3b.rearrange("t (dh w) -> t dh w", dh=SA)[
                            :, :, wb * SB:(wb + 1) * SB]
                        srca = tpa.rearrange("t (dh dw) -> t dh dw", dh=SA)
                        srcb = tpb.rearrange("t (dh dw) -> t dh dw", dh=SA)
                        if wb % 2 == 0:
                            nc.scalar.copy(dsta, srca)
                            nc.vector.tensor_copy(dstb, srcb)
                        else:
                            nc.vector.tensor_copy(dsta, srca)
                            nc.scalar.copy(dstb, srcb)

                    # final output write: 512B runs, 2 DMAs per stripe
                    nc.sync.dma_start(
                        bass.AP(out_d, h0 * W, [[H * W, TA], [W, SA], [1, W]]),
                        o3a.rearrange("t (dh w) -> t dh w", dh=SA))
                    nc.sync.dma_start(
                        bass.AP(out_d, TA * H * W + h0 * W, [[H * W, TB], [W, SA], [1, W]]),
                        o3b.rearrange("t (dh w) -> t dh w", dh=SA))

    nc.compile()
    return nc


def _get_nc():
    if "nc" not in _cache:
        _cache["nc"] = _build()
    return _cache["nc"]


def kernel(z_t: np.ndarray, z_t1: np.ndarray) -> np.ndarray:
    from concourse.bass_utils import run_bass_kernel_spmd

    nc = _get_nc()
    z_t = np.ascontiguousarray(z_t, dtype=np.float32)
    z_t1 = np.ascontiguousarray(z_t1, dtype=np.float32)
    B = z_t.shape[0]
    in_maps = [{"z_t": z_t[i], "z_t1": z_t1[i]} for i in range(B)]
    res = run_bass_kernel_spmd(nc, in_maps, core_ids=list(range(B)))
    return np.stack([res.results[i]["out"] for i in range(B)], axis=0)
